# revision 2
# baseline (speedup 1.0000x reference)
"""Two-layer GAT (GATConv 128->64x4 concat, relu, GATConv 256->2) on 8 TRN2
NeuronCores, self-contained.

v2: minimizes per-execute host->device upload (the dominant cost through the
axon tunnel) and device gather traffic.

Sharding: nodes are split 6250/core (padded to 6272 = 49 windows of 128).
Edges are partitioned by destination node; each core owns the edges whose dst
falls in its node range. Per core upload: xT shard fp16 [128, 6272] (1.6MB),
one int16 src-index stream shared by both layers (0.24MB, replicated to the 8
Q7 groups on device), uint8 slot stream (0.12MB), small weights.

Device pipeline per core:
  Phase A: h-slice[own 6272 nodes] = [x@W1 (256) | al_src (4) | al_dst (4) |
           pad] fp16, written to l1own; al_dst kept in SBUF per window.
  AllGather l1own -> l1full [50176, 384] fp16 (768B rows).
  Phase B: per window: dma_gather l1full rows by edge src; e = al_s[src] +
           al_d[dst] (dst side via PE-transposed one-hot matmul against the
           window's own al_d column); p = exp(leaky_relu(e));
           agg = onehot^T @ [h[src]*p | p] in PSUM; out1 = num/den + b1;
           relu; h2lite = relu1 @ [W2|W2 a_s2|W2 a_d2] -> l2own fp16.
  AllGather l2own -> l2full [50176, 128] fp16 (256B rows).
  Phase C: same window structure for layer 2; output [6272, 2] f32 rows.
"""

import os
import sys
import time

sys.path.insert(0, "/opt/trn_rl_repo")

import numpy as np

import concourse.bacc as bacc
import concourse.mybir as mybir
import concourse.tile as tile
from concourse.library_config import mlp
from concourse.masks import make_identity

# problem constants (hardcoded per harness contract)
N = 50000
INCH = 128
HID = 64
HEADS = 4
OUT = 2
NEG = 0.2
CORES = 8
NPC = N // CORES          # 6250 dst nodes per core
P = 128
W = 49                    # windows of 128 dst nodes per core (49*128 = 6272)
NPCP = W * P              # padded nodes per core (6272)
NROWS = CORES * NPCP      # table rows (50176)
T1C = 384                 # l1 table cols fp16 (768B rows): 0:256 h, 256:260
                          # al_src, 260:264 al_dst, rest zero
T2C = 128                 # l2 table cols fp16 (256B rows): 0:2 out, 2 al_s2,
                          # 3 al_d2, rest zero
BIAS = 32768              # int16 gather index bias
EPS = 1e-16
NQ = 4                    # SWDGE queues

f32 = mybir.dt.float32
f16 = mybir.dt.float16
i16 = mybir.dt.int16
i32 = mybir.dt.int32
u8 = mybir.dt.uint8

LAST_EXEC_NS = None
_cache = {}


def _wrap_idx_stream(arr):
    """arr [W, C] int16 -> [16, W*C//16] per-window 16-partition wrap."""
    Wn, C = arr.shape
    return arr.reshape(Wn, C // 16, 16).transpose(2, 0, 1).reshape(16, Wn * (C // 16)).copy()


def _chunks(K):
    """[(tile_off, ntiles)] with ntiles <= 8 (1024-idx dma_gather limit)."""
    out = []
    off = 0
    while off < K:
        n = min(8, K - off)
        out.append((off, n))
        off += n
    return out


def _build(K):
    C = K * P
    phases = os.environ.get("KPHASES", "ABGC")
    reps = int(os.environ.get("KREPS", "1"))
    nc = bacc.Bacc("TRN2", target_bir_lowering=False, debug=False,
                   num_devices=CORES, num_swdge_queues=NQ)

    # inputs
    xT_d = nc.dram_tensor("xT", [INCH, NPCP], f16, kind="ExternalInput")
    wcat_d = nc.dram_tensor("wcat", [INCH, T1C], f16, kind="ExternalInput")
    w2cat_d = nc.dram_tensor("w2cat", [P, 8], f32, kind="ExternalInput")
    b1_d = nc.dram_tensor("b1", [1, 256], f32, kind="ExternalInput")
    b2_d = nc.dram_tensor("b2", [1, 2], f32, kind="ExternalInput")
    idx16_d = nc.dram_tensor("idx16", [16, W * C // 16], i16, kind="ExternalInput")
    slots8_d = nc.dram_tensor("slots8", [P, W * K], u8, kind="ExternalInput")

    out_d = nc.dram_tensor("out", [NPCP, OUT], f32, kind="ExternalOutput")

    # scratch
    l1own = nc.dram_tensor("l1own", [NPCP, T1C], f16)
    l1full = nc.dram_tensor("l1full", [NROWS, T1C], f16, addr_space="Shared")
    l2own = nc.dram_tensor("l2own", [NPCP, T2C], f16)
    l2full = nc.dram_tensor("l2full", [NROWS, T2C], f16, addr_space="Shared")

    LR = mybir.AluOpType
    AF = mybir.ActivationFunctionType

    qctr = [0]

    def next_q():
        q = qctr[0] % NQ
        qctr[0] += 1
        return q

    with tile.TileContext(nc) as tc:
        with tc.tile_pool(name="const", bufs=1) as cpool:
            nc.gpsimd.load_library(mlp)

            ident_h = cpool.tile([P, P], f16)
            make_identity(nc, ident_h[:])
            ident_f = cpool.tile([P, P], f32)
            make_identity(nc, ident_f[:])
            iota_i = cpool.tile([P, P], i32)
            nc.gpsimd.iota(iota_i[:], pattern=[[1, P]], base=0, channel_multiplier=0)
            iota_h = cpool.tile([P, P], f16)
            nc.vector.tensor_copy(iota_h[:], iota_i[:])
            ones = cpool.tile([1, P], f32)
            nc.vector.memset(ones[:], 1.0)

            xT_sb = cpool.tile([INCH, NPCP], f16)
            nc.sync.dma_start(out=xT_sb[:], in_=xT_d[:, :])
            wcat_sb = cpool.tile([INCH, T1C], f16)
            nc.sync.dma_start(out=wcat_sb[:], in_=wcat_d[:, :])
            w2cat_sb = cpool.tile([P, 8], f32)
            nc.sync.dma_start(out=w2cat_sb[:], in_=w2cat_d[:, :])
            b1row = cpool.tile([1, 256], f32)
            nc.sync.dma_start(out=b1row[:], in_=b1_d[:, :])
            b2row = cpool.tile([1, 2], f32)
            nc.sync.dma_start(out=b2row[:], in_=b2_d[:, :])

            idx_sb = cpool.tile([P, W * C // 16], i16)
            for g in range(8):
                nc.sync.dma_start(
                    out=idx_sb[g * 16 : (g + 1) * 16, :], in_=idx16_d[:, :]
                )
            slots8_sb = cpool.tile([P, W * K], u8)
            nc.sync.dma_start(out=slots8_sb[:], in_=slots8_d[:, :])
            slots_f = cpool.tile([P, W * K], f32)
            nc.vector.tensor_copy(slots_f[:], slots8_sb[:])

            ald_all = cpool.tile([P, 4 * W], f16)
            ald2_all = cpool.tile([P, W], f16)
            l2stage = cpool.tile([P, T2C], f16)
            nc.vector.memset(l2stage[:], 0.0)

            # replicated biases
            with tc.tile_pool(name="psum_b", bufs=1, space="PSUM") as psb:
                b1_ps = psb.tile([P, 256], f32, space="PSUM")
                nc.tensor.matmul(out=b1_ps[:], lhsT=ones[:], rhs=b1row[:], start=True, stop=True)
                b1_rep = cpool.tile([P, 256], f32)
                nc.scalar.copy(b1_rep[:], b1_ps[:])
                b2_ps = psb.tile([P, 2], f32, space="PSUM")
                nc.tensor.matmul(out=b2_ps[:], lhsT=ones[:], rhs=b2row[:], start=True, stop=True)
                b2_rep = cpool.tile([P, 2], f32)
                nc.scalar.copy(b2_rep[:], b2_ps[:])

            for _rep in range(reps):
              # ---------------- Phase A: own-node features ----------------
              if "A" in phases:
                  with (
                      tc.tile_pool(name="sbufA", bufs=3) as pa,
                      tc.tile_pool(name="psumA", bufs=3, space="PSUM") as ppa,
                  ):
                      for w in range(W):
                          h_ps = ppa.tile([P, T1C], f32, space="PSUM", tag="h")
                          nc.tensor.matmul(
                              out=h_ps[:],
                              lhsT=xT_sb[:, w * P : (w + 1) * P],
                              rhs=wcat_sb[:],
                              start=True, stop=True,
                          )
                          stg = pa.tile([P, T1C], f16, tag="stg")
                          nc.scalar.copy(stg[:], h_ps[:])
                          nc.vector.tensor_copy(
                              ald_all[:, 4 * w : 4 * w + 4], h_ps[:, 260:264]
                          )
                          nc.sync.dma_start(
                              out=l1own[w * P : (w + 1) * P, :], in_=stg[:]
                          )

              # ---------------- AllGather layer-1 table ----------------
              if "G" in phases:
                  nc.gpsimd.collective_compute(
                      "AllGather",
                      mybir.AluOpType.bypass,
                      replica_groups=[list(range(CORES))],
                      ins=[l1own.ap().opt()],
                      outs=[l1full.ap().opt()],
                  )

              # ---------------- Phase B: layer-1 edge aggregation ----------------
              if "B" in phases:
                  with (
                      tc.tile_pool(name="sbufB", bufs=2) as pb,
                      tc.tile_pool(name="sbufBs", bufs=4) as pbs,
                      tc.tile_pool(name="psumAgg", bufs=2, space="PSUM") as pagg,
                      tc.tile_pool(name="psumT", bufs=2, space="PSUM") as pt,
                      tc.tile_pool(name="psumE", bufs=2, space="PSUM") as pe,
                      tc.tile_pool(name="psumH", bufs=1, space="PSUM") as ph,
                  ):
                      for w in range(W):
                          gbuf = pb.tile([P, K, T1C], f16, tag="gbuf")
                          for (toff, ntl) in _chunks(K):
                              nc.gpsimd.dma_gather(
                                  gbuf[:, toff : toff + ntl, :],
                                  l1full[BIAS:, :],
                                  idx_sb[:, w * (C // 16) + toff * 8 : w * (C // 16) + (toff + ntl) * 8],
                                  ntl * P,
                                  ntl * P,
                                  T1C,
                                  queue_num=next_q(),
                              )
                          agg_ps = pagg.tile([P, 260], f32, space="PSUM", tag="agg")
                          for k in range(K):
                              onehot = pbs.tile([P, P], f16, tag="onehot")
                              nc.vector.tensor_scalar(
                                  out=onehot[:],
                                  in0=iota_h[:],
                                  scalar1=slots_f[:, w * K + k : w * K + k + 1],
                                  scalar2=None,
                                  op0=LR.is_equal,
                              )
                              ohT_ps = pt.tile([P, P], f16, space="PSUM", tag="trans")
                              nc.tensor.transpose(
                                  out=ohT_ps[:], in_=onehot[:], identity=ident_h[:]
                              )
                              ohT = pbs.tile([P, P], f16, tag="ohT")
                              nc.scalar.copy(ohT[:], ohT_ps[:])
                              ed_ps = pe.tile([P, 4], f32, space="PSUM", tag="ed")
                              nc.tensor.matmul(
                                  out=ed_ps[:], lhsT=ohT[:],
                                  rhs=ald_all[:, 4 * w : 4 * w + 4],
                                  start=True, stop=True,
                              )
                              e_sb = pbs.tile([P, 4], f32, tag="e")
                              nc.vector.tensor_copy(e_sb[:], gbuf[:, k, 256:260])
                              nc.vector.tensor_tensor(
                                  out=e_sb[:], in0=e_sb[:], in1=ed_ps[:], op=LR.add
                              )
                              lr_sb = pbs.tile([P, 4], f32, tag="lr")
                              nc.vector.scalar_tensor_tensor(
                                  out=lr_sb[:], in0=e_sb[:], scalar=NEG, in1=e_sb[:],
                                  op0=LR.mult, op1=LR.max,
                              )
                              p_sb = pbs.tile([P, 4], f32, tag="p")
                              nc.scalar.activation(p_sb[:], lr_sb[:], AF.Exp)
                              msg = pbs.tile([P, 260], f16, tag="msg")
                              for h in range(HEADS):
                                  nc.scalar.mul(
                                      msg[:, h * HID : (h + 1) * HID],
                                      gbuf[:, k, h * HID : (h + 1) * HID],
                                      p_sb[:, h : h + 1],
                                  )
                              nc.vector.tensor_copy(msg[:, 256:260], p_sb[:])
                              nc.tensor.matmul(
                                  out=agg_ps[:], lhsT=onehot[:], rhs=msg[:],
                                  start=(k == 0), stop=(k == K - 1),
                              )
                          # window readout
                          den = pbs.tile([P, 4], f32, tag="den")
                          nc.vector.tensor_scalar(
                              out=den[:], in0=agg_ps[:, 256:260], scalar1=EPS,
                              scalar2=None, op0=LR.add,
                          )
                          rec = pbs.tile([P, 4], f32, tag="rec")
                          nc.vector.reciprocal(rec[:], den[:])
                          relu1 = pbs.tile([P, 256], f32, tag="relu1")
                          for h in range(HEADS):
                              nc.scalar.mul(
                                  relu1[:, h * HID : (h + 1) * HID],
                                  agg_ps[:, h * HID : (h + 1) * HID],
                                  rec[:, h : h + 1],
                              )
                          nc.vector.tensor_tensor(
                              out=relu1[:], in0=relu1[:], in1=b1_rep[:], op=LR.add
                          )
                          nc.scalar.activation(relu1[:], relu1[:], AF.Relu)
                          h2_ps = ph.tile([P, 4], f32, space="PSUM", tag="h2")
                          for half in range(2):
                              rT_ps = pt.tile([P, P], f32, space="PSUM", tag="trans")
                              nc.tensor.transpose(
                                  out=rT_ps[:], in_=relu1[:, half * P : (half + 1) * P],
                                  identity=ident_f[:],
                              )
                              rT = pbs.tile([P, P], f32, tag="rT")
                              nc.scalar.copy(rT[:], rT_ps[:])
                              nc.tensor.matmul(
                                  out=h2_ps[:], lhsT=rT[:],
                                  rhs=w2cat_sb[:, half * 4 : (half + 1) * 4],
                                  start=(half == 0), stop=(half == 1),
                              )
                          nc.vector.tensor_copy(l2stage[:, 0:4], h2_ps[:])
                          nc.vector.tensor_copy(ald2_all[:, w : w + 1], h2_ps[:, 3:4])
                          nc.sync.dma_start(
                              out=l2own[w * P : (w + 1) * P, :], in_=l2stage[:]
                          )

              # ---------------- AllGather layer-2 table ----------------
              if "G" in phases:
                  nc.gpsimd.collective_compute(
                      "AllGather",
                      mybir.AluOpType.bypass,
                      replica_groups=[list(range(CORES))],
                      ins=[l2own.ap().opt()],
                      outs=[l2full.ap().opt()],
                  )

              # ---------------- Phase C: layer-2 edge aggregation ----------------
              if "C" in phases:
                  with (
                      tc.tile_pool(name="sbufC", bufs=2) as pc,
                      tc.tile_pool(name="sbufCs", bufs=4) as pcs,
                      tc.tile_pool(name="psumAgg2", bufs=2, space="PSUM") as pagg2,
                      tc.tile_pool(name="psumT2", bufs=2, space="PSUM") as pt2,
                      tc.tile_pool(name="psumE2", bufs=2, space="PSUM") as pe2,
                  ):
                      for w in range(W):
                          g2 = pc.tile([P, K, T2C], f16, tag="g2")
                          for (toff, ntl) in _chunks(K):
                              nc.gpsimd.dma_gather(
                                  g2[:, toff : toff + ntl, :],
                                  l2full[BIAS:, :],
                                  idx_sb[:, w * (C // 16) + toff * 8 : w * (C // 16) + (toff + ntl) * 8],
                                  ntl * P,
                                  ntl * P,
                                  T2C,
                                  queue_num=next_q(),
                              )
                          agg2_ps = pagg2.tile([P, 3], f32, space="PSUM", tag="agg2")
                          for k in range(K):
                              onehot = pcs.tile([P, P], f16, tag="onehot2")
                              nc.vector.tensor_scalar(
                                  out=onehot[:],
                                  in0=iota_h[:],
                                  scalar1=slots_f[:, w * K + k : w * K + k + 1],
                                  scalar2=None,
                                  op0=LR.is_equal,
                              )
                              ohT_ps = pt2.tile([P, P], f16, space="PSUM", tag="trans2")
                              nc.tensor.transpose(
                                  out=ohT_ps[:], in_=onehot[:], identity=ident_h[:]
                              )
                              ohT = pcs.tile([P, P], f16, tag="ohT2")
                              nc.scalar.copy(ohT[:], ohT_ps[:])
                              ed_ps = pe2.tile([P, 1], f32, space="PSUM", tag="ed2")
                              nc.tensor.matmul(
                                  out=ed_ps[:], lhsT=ohT[:],
                                  rhs=ald2_all[:, w : w + 1],
                                  start=True, stop=True,
                              )
                              e_sb = pcs.tile([P, 1], f32, tag="e2")
                              nc.vector.tensor_copy(e_sb[:], g2[:, k, 2:3])
                              nc.vector.tensor_tensor(
                                  out=e_sb[:], in0=e_sb[:], in1=ed_ps[:], op=LR.add
                              )
                              lr_sb = pcs.tile([P, 1], f32, tag="lr2")
                              nc.vector.scalar_tensor_tensor(
                                  out=lr_sb[:], in0=e_sb[:], scalar=NEG, in1=e_sb[:],
                                  op0=LR.mult, op1=LR.max,
                              )
                              p_sb = pcs.tile([P, 1], f32, tag="p2")
                              nc.scalar.activation(p_sb[:], lr_sb[:], AF.Exp)
                              msg = pcs.tile([P, 3], f16, tag="msg2")
                              nc.scalar.mul(msg[:, 0:2], g2[:, k, 0:2], p_sb[:, 0:1])
                              nc.vector.tensor_copy(msg[:, 2:3], p_sb[:])
                              nc.tensor.matmul(
                                  out=agg2_ps[:], lhsT=onehot[:], rhs=msg[:],
                                  start=(k == 0), stop=(k == K - 1),
                              )
                          den = pcs.tile([P, 1], f32, tag="den2")
                          nc.vector.tensor_scalar(
                              out=den[:], in0=agg2_ps[:, 2:3], scalar1=EPS,
                              scalar2=None, op0=LR.add,
                          )
                          rec = pcs.tile([P, 1], f32, tag="rec2")
                          nc.vector.reciprocal(rec[:], den[:])
                          o2 = pcs.tile([P, OUT], f32, tag="o2")
                          nc.scalar.mul(o2[:], agg2_ps[:, 0:2], rec[:, 0:1])
                          nc.vector.tensor_tensor(out=o2[:], in0=o2[:], in1=b2_rep[:], op=LR.add)
                          nc.sync.dma_start(out=out_d[w * P : (w + 1) * P, :], in_=o2[:])

    nc.compile()
    return nc


def _preprocess(x, edge_index, W1, a_src1, a_dst1, b1, W2, a_src2, a_dst2, b2):
    src = np.concatenate([np.asarray(edge_index[0]), np.arange(N)]).astype(np.int64)
    dst = np.concatenate([np.asarray(edge_index[1]), np.arange(N)]).astype(np.int64)

    core = dst // NPC
    loc = dst - core * NPC
    win = loc >> 7
    slot = loc & 127
    gid = core * W + win
    order = np.argsort(gid, kind="stable")
    counts = np.bincount(gid, minlength=CORES * W)
    K = int(np.ceil((counts.max() + 1) / P))
    while counts.max() > K * P - len(_chunks(K)):
        K += 1
    C = K * P

    starts = np.zeros(CORES * W, np.int64)
    starts[1:] = np.cumsum(counts)[:-1]
    within = np.arange(len(order)) - starts[gid[order]]
    # map within-window rank -> position, skipping the reserved last slot of
    # each gather chunk (keeps every chunk's final index non-negative)
    usable = np.array([n * P - 1 for (_, n) in _chunks(K)], np.int64)
    cumu = np.cumsum(usable)
    ci = np.searchsorted(cumu, within, side="right")
    pos = gid[order] * C + within + ci

    s_sorted = src[order]
    perm = (s_sorted // NPC) * NPCP + (s_sorted % NPC)
    idx = np.full(CORES * W * C, BIAS, np.int64)
    idx[pos] = perm
    idx = (idx - BIAS).astype(np.int16).reshape(CORES, W, C)
    slotv = np.full(CORES * W * C, P, np.uint8)
    slotv[pos] = slot[order].astype(np.uint8)
    slotv = slotv.reshape(CORES, W, K, P)

    # weights
    W1 = np.asarray(W1, np.float32)
    W1r = W1.reshape(INCH, HEADS, HID)
    wa_s = np.einsum("ihc,hc->ih", W1r, np.asarray(a_src1, np.float32))
    wa_d = np.einsum("ihc,hc->ih", W1r, np.asarray(a_dst1, np.float32))
    wcat = np.zeros((INCH, T1C), np.float16)
    wcat[:, 0:256] = W1.astype(np.float16)
    wcat[:, 256:260] = wa_s.astype(np.float16)
    wcat[:, 260:264] = wa_d.astype(np.float16)

    W2 = np.asarray(W2, np.float32)
    w2s = W2 @ np.asarray(a_src2, np.float32)[0]
    w2d = W2 @ np.asarray(a_dst2, np.float32)[0]
    w2cat = np.concatenate([W2, w2s[:, None], w2d[:, None]], axis=1).astype(np.float32)
    w2cat = np.concatenate([w2cat[:P], w2cat[P:]], axis=1)  # [128, 8]

    x = np.asarray(x, np.float32)
    in_maps = []
    for c in range(CORES):
        xs = np.zeros((NPCP, INCH), np.float16)
        xs[:NPC] = x[c * NPC : (c + 1) * NPC].astype(np.float16)
        in_maps.append(
            {
                "xT": np.ascontiguousarray(xs.T),
                "wcat": wcat,
                "w2cat": w2cat,
                "b1": np.asarray(b1, np.float32).reshape(1, 256),
                "b2": np.asarray(b2, np.float32).reshape(1, 2),
                "idx16": _wrap_idx_stream(idx[c]),
                "slots8": slotv[c].transpose(2, 0, 1).reshape(P, W * K).copy(),
            }
        )
    return K, in_maps


class _Runner:
    """Persistent compiled runner: jit once, device-resident inputs, so
    repeated calls time only execution (+ dispatch)."""

    def __init__(self, nc):
        import jax
        from jax.sharding import Mesh, PartitionSpec, NamedSharding
        from jax.experimental.shard_map import shard_map
        from concourse import bass2jax
        import concourse.mybir as mb

        bass2jax.install_neuronx_cc_hook()
        self.jax = jax
        self.nc = nc
        part_name = nc.partition_id_tensor.name if nc.partition_id_tensor else None
        in_names, out_names, out_avals, zero_outs = [], [], [], []
        for alloc in nc.m.functions[0].allocations:
            if not isinstance(alloc, mb.MemoryLocationSet):
                continue
            name = alloc.memorylocations[0].name
            if alloc.kind == "ExternalInput":
                if name != part_name:
                    in_names.append(name)
            elif alloc.kind == "ExternalOutput":
                out_names.append(name)
                shape = tuple(alloc.tensor_shape)
                dtype = mb.dt.np(alloc.dtype)
                out_avals.append(jax.core.ShapedArray(shape, dtype))
                zero_outs.append(np.zeros(shape, dtype))
        self.in_names, self.out_names = in_names, out_names
        self.zero_outs = zero_outs
        n_params, n_outs = len(in_names), len(out_names)
        donate = tuple(range(n_params, n_params + n_outs))

        all_in_names = in_names + out_names + ([part_name] if part_name else [])

        def _body(*args):
            operands = list(args)
            if part_name is not None:
                operands.append(bass2jax.partition_id_tensor())
            outs = bass2jax._bass_exec_p.bind(
                *operands,
                out_avals=tuple(out_avals),
                in_names=tuple(all_in_names),
                out_names=tuple(out_names),
                lowering_input_output_aliases=(),
                sim_require_finite=True,
                sim_require_nnan=True,
                nc=nc,
            )
            return tuple(outs)

        devices = jax.devices()[:CORES]
        self.mesh = Mesh(np.asarray(devices), ("core",))
        self.spec = NamedSharding(self.mesh, PartitionSpec("core"))
        in_specs = (PartitionSpec("core"),) * (n_params + n_outs)
        out_specs = (PartitionSpec("core"),) * n_outs
        self.sharded = jax.jit(
            shard_map(_body, mesh=self.mesh, in_specs=in_specs,
                      out_specs=out_specs, check_rep=False),
            donate_argnums=donate, keep_unused=True,
        )
        self.dev_in = None

    def put_inputs(self, in_maps):
        self.dev_in = [
            self.jax.device_put(
                np.concatenate([np.asarray(m[n]) for m in in_maps], axis=0), self.spec
            )
            for n in self.in_names
        ]

    def execute(self):
        zeros = [
            self.jax.device_put(
                np.zeros((CORES * z.shape[0], *z.shape[1:]), z.dtype), self.spec
            )
            for z in self.zero_outs
        ]
        for z in zeros:
            z.block_until_ready()
        t0 = time.monotonic_ns()
        outs = self.sharded(*self.dev_in, *zeros)
        for o in outs:
            o.block_until_ready()
        dt = time.monotonic_ns() - t0
        res = [
            {
                name: np.asarray(outs[i]).reshape(CORES, *self.zero_outs[i].shape)[c]
                for i, name in enumerate(self.out_names)
            }
            for c in range(CORES)
        ]
        return res, dt


def run_on_device(in_maps, K):
    if K not in _cache:
        _cache[K] = _Runner(_build(K))
    runner = _cache[K]
    runner.put_inputs(in_maps)
    res, dt = runner.execute()
    global LAST_EXEC_NS
    LAST_EXEC_NS = dt
    return res


def kernel(x, edge_index, W1, a_src1, a_dst1, b1, W2, a_src2, a_dst2, b2):
    global LAST_EXEC_NS
    K, in_maps = _preprocess(
        x, edge_index, W1, a_src1, a_dst1, b1, W2, a_src2, a_dst2, b2
    )
    res = run_on_device(in_maps, K)
    out = np.concatenate([res[c]["out"][:NPC] for c in range(CORES)], axis=0)
    return out.astype(np.float32)


# revision 3
# speedup vs baseline: 1.0337x; 1.0337x over previous
"""Two-layer GAT (GATConv 128->64x4 concat, relu, GATConv 256->2) on 8 TRN2
NeuronCores, self-contained.

v2: minimizes per-execute host->device upload (the dominant cost through the
axon tunnel) and device gather traffic.

Sharding: nodes are split 6250/core (padded to 6272 = 49 windows of 128).
Edges are partitioned by destination node; each core owns the edges whose dst
falls in its node range. Per core upload: xT shard fp16 [128, 6272] (1.6MB),
one int16 src-index stream shared by both layers (0.24MB, replicated to the 8
Q7 groups on device), uint8 slot stream (0.12MB), small weights.

Device pipeline per core:
  Phase A: h-slice[own 6272 nodes] = [x@W1 (256) | al_src (4) | al_dst (4) |
           pad] fp16, written to l1own; al_dst kept in SBUF per window.
  AllGather l1own -> l1full [50176, 384] fp16 (768B rows).
  Phase B: per window: dma_gather l1full rows by edge src; e = al_s[src] +
           al_d[dst] (dst side via PE-transposed one-hot matmul against the
           window's own al_d column); p = exp(leaky_relu(e));
           agg = onehot^T @ [h[src]*p | p] in PSUM; out1 = num/den + b1;
           relu; h2lite = relu1 @ [W2|W2 a_s2|W2 a_d2] -> l2own fp16.
  AllGather l2own -> l2full [50176, 128] fp16 (256B rows).
  Phase C: same window structure for layer 2; output [6272, 2] f32 rows.
"""

import os
import sys
import time

sys.path.insert(0, "/opt/trn_rl_repo")

import numpy as np

import concourse.bacc as bacc
import concourse.mybir as mybir
import concourse.tile as tile
from concourse.library_config import mlp
from concourse.masks import make_identity

# problem constants (hardcoded per harness contract)
N = 50000
INCH = 128
HID = 64
HEADS = 4
OUT = 2
NEG = 0.2
CORES = 8
NPC = N // CORES          # 6250 dst nodes per core
P = 128
W = 49                    # windows of 128 dst nodes per core (49*128 = 6272)
NPCP = W * P              # padded nodes per core (6272)
NROWS = CORES * NPCP      # table rows (50176)
T1C = 384                 # l1 table cols fp16 (768B rows): 0:256 h, 256:260
                          # al_src, 260:264 al_dst, rest zero
T2C = 128                 # l2 table cols fp16 (256B rows): 0:2 out, 2 al_s2,
                          # 3 al_d2, rest zero
BIAS = 32768              # int16 gather index bias
EPS = 1e-16
NQ = 4                    # SWDGE queues

f32 = mybir.dt.float32
f16 = mybir.dt.float16
i16 = mybir.dt.int16
i32 = mybir.dt.int32
u8 = mybir.dt.uint8

LAST_EXEC_NS = None
_cache = {}


def _wrap_idx_stream(arr):
    """arr [W, C] int16 -> [16, W*C//16] per-window 16-partition wrap."""
    Wn, C = arr.shape
    return arr.reshape(Wn, C // 16, 16).transpose(2, 0, 1).reshape(16, Wn * (C // 16)).copy()


def _chunks(K):
    """[(tile_off, ntiles)] with ntiles <= 8 (1024-idx dma_gather limit)."""
    out = []
    off = 0
    while off < K:
        n = min(8, K - off)
        out.append((off, n))
        off += n
    return out


def _build(K):
    C = K * P
    phases = os.environ.get("KPHASES", "ABGC")
    reps = int(os.environ.get("KREPS", "1"))
    nc = bacc.Bacc("TRN2", target_bir_lowering=False, debug=False,
                   num_devices=CORES, num_swdge_queues=NQ)

    # inputs
    xT_d = nc.dram_tensor("xT", [INCH, NPCP], f16, kind="ExternalInput")
    wcat_d = nc.dram_tensor("wcat", [INCH, T1C], f16, kind="ExternalInput")
    w2cat_d = nc.dram_tensor("w2cat", [P, 8], f16, kind="ExternalInput")
    b1_d = nc.dram_tensor("b1", [1, 256], f32, kind="ExternalInput")
    b2_d = nc.dram_tensor("b2", [1, 2], f32, kind="ExternalInput")
    idx16_d = nc.dram_tensor("idx16", [16, W * C // 16], i16, kind="ExternalInput")
    slots8_d = nc.dram_tensor("slots8", [P, W * K], u8, kind="ExternalInput")

    out_d = nc.dram_tensor("out", [NPCP, OUT], f32, kind="ExternalOutput")

    # scratch
    l1own = nc.dram_tensor("l1own", [NPCP, T1C], f16)
    l1full = nc.dram_tensor("l1full", [NROWS, T1C], f16, addr_space="Shared")
    l2own = nc.dram_tensor("l2own", [NPCP, T2C], f16)
    l2full = nc.dram_tensor("l2full", [NROWS, T2C], f16, addr_space="Shared")

    LR = mybir.AluOpType
    AF = mybir.ActivationFunctionType

    qctr = [0]

    def next_q():
        q = qctr[0] % NQ
        qctr[0] += 1
        return q

    with tile.TileContext(nc) as tc:
        with tc.tile_pool(name="const", bufs=1) as cpool:
            nc.gpsimd.load_library(mlp)

            ident_h = cpool.tile([P, P], f16)
            make_identity(nc, ident_h[:])
            ident_f = cpool.tile([P, P], f32)
            make_identity(nc, ident_f[:])
            iota_i = cpool.tile([P, P], i32)
            nc.gpsimd.iota(iota_i[:], pattern=[[1, P]], base=0, channel_multiplier=0)
            iota_h = cpool.tile([P, P], f16)
            nc.vector.tensor_copy(iota_h[:], iota_i[:])
            ones = cpool.tile([1, P], f32)
            nc.vector.memset(ones[:], 1.0)

            xT_sb = cpool.tile([INCH, NPCP], f16)
            nc.sync.dma_start(out=xT_sb[:], in_=xT_d[:, :])
            wcat_sb = cpool.tile([INCH, T1C], f16)
            nc.sync.dma_start(out=wcat_sb[:], in_=wcat_d[:, :])
            w2cat_sb = cpool.tile([P, 8], f16)
            nc.sync.dma_start(out=w2cat_sb[:], in_=w2cat_d[:, :])
            b1row = cpool.tile([1, 256], f32)
            nc.sync.dma_start(out=b1row[:], in_=b1_d[:, :])
            b2row = cpool.tile([1, 2], f32)
            nc.sync.dma_start(out=b2row[:], in_=b2_d[:, :])

            idx_sb = cpool.tile([P, W * C // 16], i16)
            for g in range(8):
                nc.sync.dma_start(
                    out=idx_sb[g * 16 : (g + 1) * 16, :], in_=idx16_d[:, :]
                )
            slots8_sb = cpool.tile([P, W * K], u8)
            nc.sync.dma_start(out=slots8_sb[:], in_=slots8_d[:, :])
            slots_f = cpool.tile([P, W * K], f32)
            nc.vector.tensor_copy(slots_f[:], slots8_sb[:])

            ald_all = cpool.tile([P, 4 * W], f16)
            ald2_all = cpool.tile([P, W], f16)
            ed2_all = cpool.tile([P, W * K], f16)
            l2stage = cpool.tile([P, T2C], f16)
            nc.vector.memset(l2stage[:], 0.0)

            # replicated biases
            with tc.tile_pool(name="psum_b", bufs=1, space="PSUM") as psb:
                b1_ps = psb.tile([P, 256], f32, space="PSUM")
                nc.tensor.matmul(out=b1_ps[:], lhsT=ones[:], rhs=b1row[:], start=True, stop=True)
                b1_rep = cpool.tile([P, 256], f32)
                nc.scalar.copy(b1_rep[:], b1_ps[:])
                b2_ps = psb.tile([P, 2], f32, space="PSUM")
                nc.tensor.matmul(out=b2_ps[:], lhsT=ones[:], rhs=b2row[:], start=True, stop=True)
                b2_rep = cpool.tile([P, 2], f32)
                nc.scalar.copy(b2_rep[:], b2_ps[:])

            for _rep in range(reps):
              # ---------------- Phase A: own-node features ----------------
              if "A" in phases:
                  with (
                      tc.tile_pool(name="sbufA", bufs=3) as pa,
                      tc.tile_pool(name="psumA", bufs=3, space="PSUM") as ppa,
                  ):
                      for w in range(W):
                          h_ps = ppa.tile([P, T1C], f32, space="PSUM", tag="h")
                          nc.tensor.matmul(
                              out=h_ps[:],
                              lhsT=xT_sb[:, w * P : (w + 1) * P],
                              rhs=wcat_sb[:],
                              start=True, stop=True,
                          )
                          stg = pa.tile([P, T1C], f16, tag="stg")
                          nc.scalar.copy(stg[:], h_ps[:])
                          nc.vector.tensor_copy(
                              ald_all[:, 4 * w : 4 * w + 4], h_ps[:, 260:264]
                          )
                          nc.sync.dma_start(
                              out=l1own[w * P : (w + 1) * P, :], in_=stg[:]
                          )

              # ---------------- AllGather layer-1 table ----------------
              if "G" in phases:
                  nc.gpsimd.collective_compute(
                      "AllGather",
                      mybir.AluOpType.bypass,
                      replica_groups=[list(range(CORES))],
                      ins=[l1own.ap().opt()],
                      outs=[l1full.ap().opt()],
                  )

              # ---------------- Phase B: layer-1 edge aggregation ----------------
              if "B" in phases:
                  with (
                      tc.tile_pool(name="sbufB", bufs=3) as pb,
                      tc.tile_pool(name="sbufBs", bufs=6) as pbs,
                      tc.tile_pool(name="psumAgg", bufs=2, space="PSUM") as pagg,
                      tc.tile_pool(name="psumT", bufs=2, space="PSUM") as pt,
                      tc.tile_pool(name="psumE", bufs=2, space="PSUM") as pe,
                      tc.tile_pool(name="psumH", bufs=1, space="PSUM") as ph,
                  ):
                      for w in range(W):
                          gbuf = pb.tile([P, K, T1C], f16, tag="gbuf")
                          ohTall = pb.tile([P, K, P], f16, tag="ohTall")
                          for (toff, ntl) in _chunks(K):
                              nc.gpsimd.dma_gather(
                                  gbuf[:, toff : toff + ntl, :],
                                  l1full[BIAS:, :],
                                  idx_sb[:, w * (C // 16) + toff * 8 : w * (C // 16) + (toff + ntl) * 8],
                                  ntl * P,
                                  ntl * P,
                                  T1C,
                                  queue_num=next_q(),
                              )
                          agg_ps = pagg.tile([P, 260], f32, space="PSUM", tag="agg")
                          for k in range(K):
                              onehot = pbs.tile([P, P], f16, tag="onehot")
                              nc.vector.tensor_scalar(
                                  out=onehot[:],
                                  in0=iota_h[:],
                                  scalar1=slots_f[:, w * K + k : w * K + k + 1],
                                  scalar2=None,
                                  op0=LR.is_equal,
                              )
                              ohT_ps = pt.tile([P, P], f16, space="PSUM", tag="trans")
                              nc.tensor.transpose(
                                  out=ohT_ps[:], in_=onehot[:], identity=ident_h[:]
                              )
                              nc.scalar.copy(ohTall[:, k, :], ohT_ps[:])
                              ed_ps = pe.tile([P, 4], f32, space="PSUM", tag="ed")
                              nc.tensor.matmul(
                                  out=ed_ps[:], lhsT=ohTall[:, k, :],
                                  rhs=ald_all[:, 4 * w : 4 * w + 4],
                                  start=True, stop=True,
                              )
                              e_sb = pbs.tile([P, 4], f32, tag="e")
                              nc.vector.tensor_copy(e_sb[:], gbuf[:, k, 256:260])
                              nc.vector.tensor_tensor(
                                  out=e_sb[:], in0=e_sb[:], in1=ed_ps[:], op=LR.add
                              )
                              lr_sb = pbs.tile([P, 4], f32, tag="lr")
                              nc.vector.scalar_tensor_tensor(
                                  out=lr_sb[:], in0=e_sb[:], scalar=NEG, in1=e_sb[:],
                                  op0=LR.mult, op1=LR.max,
                              )
                              p_sb = pbs.tile([P, 4], f32, tag="p")
                              nc.scalar.activation(p_sb[:], lr_sb[:], AF.Exp)
                              msg = pbs.tile([P, 260], f16, tag="msg")
                              for h in range(HEADS):
                                  nc.scalar.mul(
                                      msg[:, h * HID : (h + 1) * HID],
                                      gbuf[:, k, h * HID : (h + 1) * HID],
                                      p_sb[:, h : h + 1],
                                  )
                              nc.vector.tensor_copy(msg[:, 256:260], p_sb[:])
                              nc.tensor.matmul(
                                  out=agg_ps[:], lhsT=onehot[:], rhs=msg[:],
                                  start=(k == 0), stop=(k == K - 1),
                              )
                          # window readout
                          den = pbs.tile([P, 4], f32, tag="den")
                          nc.vector.tensor_scalar(
                              out=den[:], in0=agg_ps[:, 256:260], scalar1=EPS,
                              scalar2=None, op0=LR.add,
                          )
                          rec = pbs.tile([P, 4], f32, tag="rec")
                          nc.vector.reciprocal(rec[:], den[:])
                          relu1f = pbs.tile([P, 256], f32, tag="relu1f")
                          for h in range(HEADS):
                              nc.scalar.mul(
                                  relu1f[:, h * HID : (h + 1) * HID],
                                  agg_ps[:, h * HID : (h + 1) * HID],
                                  rec[:, h : h + 1],
                              )
                          nc.vector.tensor_tensor(
                              out=relu1f[:], in0=relu1f[:], in1=b1_rep[:], op=LR.add
                          )
                          relu1 = pbs.tile([P, 256], f16, tag="relu1")
                          nc.scalar.activation(relu1[:], relu1f[:], AF.Relu)
                          h2_ps = ph.tile([P, 4], f32, space="PSUM", tag="h2")
                          for half in range(2):
                              rT_ps = pt.tile([P, P], f16, space="PSUM", tag="trans")
                              nc.tensor.transpose(
                                  out=rT_ps[:], in_=relu1[:, half * P : (half + 1) * P],
                                  identity=ident_h[:],
                              )
                              rT = pbs.tile([P, P], f16, tag="rT")
                              nc.scalar.copy(rT[:], rT_ps[:])
                              nc.tensor.matmul(
                                  out=h2_ps[:], lhsT=rT[:],
                                  rhs=w2cat_sb[:, half * 4 : (half + 1) * 4],
                                  start=(half == 0), stop=(half == 1),
                              )
                          nc.vector.tensor_copy(l2stage[:, 0:4], h2_ps[:])
                          ald2_w = pbs.tile([P, 1], f16, tag="ald2w")
                          nc.vector.tensor_copy(ald2_w[:], h2_ps[:, 3:4])
                          nc.vector.tensor_copy(ald2_all[:, w : w + 1], ald2_w[:])
                          for k in range(K):
                              ed2_ps = pe.tile([P, 4], f32, space="PSUM", tag="ed")
                              nc.tensor.matmul(
                                  out=ed2_ps[:, 0:1], lhsT=ohTall[:, k, :],
                                  rhs=ald2_w[:],
                                  start=True, stop=True,
                              )
                              nc.vector.tensor_copy(
                                  ed2_all[:, w * K + k : w * K + k + 1], ed2_ps[:, 0:1]
                              )
                          nc.sync.dma_start(
                              out=l2own[w * P : (w + 1) * P, :], in_=l2stage[:]
                          )

              # ---------------- AllGather layer-2 table ----------------
              if "G" in phases:
                  nc.gpsimd.collective_compute(
                      "AllGather",
                      mybir.AluOpType.bypass,
                      replica_groups=[list(range(CORES))],
                      ins=[l2own.ap().opt()],
                      outs=[l2full.ap().opt()],
                  )

              # ---------------- Phase C: layer-2 edge aggregation ----------------
              if "C" in phases:
                  with (
                      tc.tile_pool(name="sbufC", bufs=3) as pc,
                      tc.tile_pool(name="sbufCs", bufs=6) as pcs,
                      tc.tile_pool(name="psumAgg2", bufs=2, space="PSUM") as pagg2,
                  ):
                      for w in range(W):
                          g2 = pc.tile([P, K, T2C], f16, tag="g2")
                          for (toff, ntl) in _chunks(K):
                              nc.gpsimd.dma_gather(
                                  g2[:, toff : toff + ntl, :],
                                  l2full[BIAS:, :],
                                  idx_sb[:, w * (C // 16) + toff * 8 : w * (C // 16) + (toff + ntl) * 8],
                                  ntl * P,
                                  ntl * P,
                                  T2C,
                                  queue_num=next_q(),
                              )
                          agg2_ps = pagg2.tile([P, 3], f32, space="PSUM", tag="agg2")
                          for k in range(K):
                              onehot = pcs.tile([P, P], f16, tag="onehot2")
                              nc.vector.tensor_scalar(
                                  out=onehot[:],
                                  in0=iota_h[:],
                                  scalar1=slots_f[:, w * K + k : w * K + k + 1],
                                  scalar2=None,
                                  op0=LR.is_equal,
                              )
                              e_sb = pcs.tile([P, 1], f32, tag="e2")
                              nc.vector.tensor_tensor(
                                  out=e_sb[:], in0=g2[:, k, 2:3],
                                  in1=ed2_all[:, w * K + k : w * K + k + 1], op=LR.add
                              )
                              lr_sb = pcs.tile([P, 1], f32, tag="lr2")
                              nc.vector.scalar_tensor_tensor(
                                  out=lr_sb[:], in0=e_sb[:], scalar=NEG, in1=e_sb[:],
                                  op0=LR.mult, op1=LR.max,
                              )
                              p_sb = pcs.tile([P, 1], f32, tag="p2")
                              nc.scalar.activation(p_sb[:], lr_sb[:], AF.Exp)
                              msg = pcs.tile([P, 3], f16, tag="msg2")
                              nc.scalar.mul(msg[:, 0:2], g2[:, k, 0:2], p_sb[:, 0:1])
                              nc.vector.tensor_copy(msg[:, 2:3], p_sb[:])
                              nc.tensor.matmul(
                                  out=agg2_ps[:], lhsT=onehot[:], rhs=msg[:],
                                  start=(k == 0), stop=(k == K - 1),
                              )
                          den = pcs.tile([P, 1], f32, tag="den2")
                          nc.vector.tensor_scalar(
                              out=den[:], in0=agg2_ps[:, 2:3], scalar1=EPS,
                              scalar2=None, op0=LR.add,
                          )
                          rec = pcs.tile([P, 1], f32, tag="rec2")
                          nc.vector.reciprocal(rec[:], den[:])
                          o2 = pcs.tile([P, OUT], f32, tag="o2")
                          nc.scalar.mul(o2[:], agg2_ps[:, 0:2], rec[:, 0:1])
                          nc.vector.tensor_tensor(out=o2[:], in0=o2[:], in1=b2_rep[:], op=LR.add)
                          nc.sync.dma_start(out=out_d[w * P : (w + 1) * P, :], in_=o2[:])

    nc.compile()
    return nc


def _preprocess(x, edge_index, W1, a_src1, a_dst1, b1, W2, a_src2, a_dst2, b2):
    src = np.concatenate([np.asarray(edge_index[0]), np.arange(N)]).astype(np.int64)
    dst = np.concatenate([np.asarray(edge_index[1]), np.arange(N)]).astype(np.int64)

    core = dst // NPC
    loc = dst - core * NPC
    win = loc >> 7
    slot = loc & 127
    gid = core * W + win
    order = np.argsort(gid, kind="stable")
    counts = np.bincount(gid, minlength=CORES * W)
    K = int(np.ceil((counts.max() + 1) / P))
    while counts.max() > K * P - len(_chunks(K)):
        K += 1
    C = K * P

    starts = np.zeros(CORES * W, np.int64)
    starts[1:] = np.cumsum(counts)[:-1]
    within = np.arange(len(order)) - starts[gid[order]]
    # map within-window rank -> position, skipping the reserved last slot of
    # each gather chunk (keeps every chunk's final index non-negative)
    usable = np.array([n * P - 1 for (_, n) in _chunks(K)], np.int64)
    cumu = np.cumsum(usable)
    ci = np.searchsorted(cumu, within, side="right")
    pos = gid[order] * C + within + ci

    s_sorted = src[order]
    perm = (s_sorted // NPC) * NPCP + (s_sorted % NPC)
    idx = np.full(CORES * W * C, BIAS, np.int64)
    idx[pos] = perm
    idx = (idx - BIAS).astype(np.int16).reshape(CORES, W, C)
    slotv = np.full(CORES * W * C, P, np.uint8)
    slotv[pos] = slot[order].astype(np.uint8)
    slotv = slotv.reshape(CORES, W, K, P)

    # weights
    W1 = np.asarray(W1, np.float32)
    W1r = W1.reshape(INCH, HEADS, HID)
    wa_s = np.einsum("ihc,hc->ih", W1r, np.asarray(a_src1, np.float32))
    wa_d = np.einsum("ihc,hc->ih", W1r, np.asarray(a_dst1, np.float32))
    wcat = np.zeros((INCH, T1C), np.float16)
    wcat[:, 0:256] = W1.astype(np.float16)
    wcat[:, 256:260] = wa_s.astype(np.float16)
    wcat[:, 260:264] = wa_d.astype(np.float16)

    W2 = np.asarray(W2, np.float32)
    w2s = W2 @ np.asarray(a_src2, np.float32)[0]
    w2d = W2 @ np.asarray(a_dst2, np.float32)[0]
    w2cat = np.concatenate([W2, w2s[:, None], w2d[:, None]], axis=1)
    w2cat = np.concatenate([w2cat[:P], w2cat[P:]], axis=1).astype(np.float16)  # [128, 8]

    x = np.asarray(x, np.float32)
    in_maps = []
    for c in range(CORES):
        xs = np.zeros((NPCP, INCH), np.float16)
        xs[:NPC] = x[c * NPC : (c + 1) * NPC].astype(np.float16)
        in_maps.append(
            {
                "xT": np.ascontiguousarray(xs.T),
                "wcat": wcat,
                "w2cat": w2cat,
                "b1": np.asarray(b1, np.float32).reshape(1, 256),
                "b2": np.asarray(b2, np.float32).reshape(1, 2),
                "idx16": _wrap_idx_stream(idx[c]),
                "slots8": slotv[c].transpose(2, 0, 1).reshape(P, W * K).copy(),
            }
        )
    return K, in_maps


class _Runner:
    """Persistent compiled runner: jit once, device-resident inputs, so
    repeated calls time only execution (+ dispatch)."""

    def __init__(self, nc):
        import jax
        from jax.sharding import Mesh, PartitionSpec, NamedSharding
        from jax.experimental.shard_map import shard_map
        from concourse import bass2jax
        import concourse.mybir as mb

        bass2jax.install_neuronx_cc_hook()
        self.jax = jax
        self.nc = nc
        part_name = nc.partition_id_tensor.name if nc.partition_id_tensor else None
        in_names, out_names, out_avals, zero_outs = [], [], [], []
        for alloc in nc.m.functions[0].allocations:
            if not isinstance(alloc, mb.MemoryLocationSet):
                continue
            name = alloc.memorylocations[0].name
            if alloc.kind == "ExternalInput":
                if name != part_name:
                    in_names.append(name)
            elif alloc.kind == "ExternalOutput":
                out_names.append(name)
                shape = tuple(alloc.tensor_shape)
                dtype = mb.dt.np(alloc.dtype)
                out_avals.append(jax.core.ShapedArray(shape, dtype))
                zero_outs.append(np.zeros(shape, dtype))
        self.in_names, self.out_names = in_names, out_names
        self.zero_outs = zero_outs
        n_params, n_outs = len(in_names), len(out_names)
        donate = tuple(range(n_params, n_params + n_outs))

        all_in_names = in_names + out_names + ([part_name] if part_name else [])

        def _body(*args):
            operands = list(args)
            if part_name is not None:
                operands.append(bass2jax.partition_id_tensor())
            outs = bass2jax._bass_exec_p.bind(
                *operands,
                out_avals=tuple(out_avals),
                in_names=tuple(all_in_names),
                out_names=tuple(out_names),
                lowering_input_output_aliases=(),
                sim_require_finite=True,
                sim_require_nnan=True,
                nc=nc,
            )
            return tuple(outs)

        devices = jax.devices()[:CORES]
        self.mesh = Mesh(np.asarray(devices), ("core",))
        self.spec = NamedSharding(self.mesh, PartitionSpec("core"))
        in_specs = (PartitionSpec("core"),) * (n_params + n_outs)
        out_specs = (PartitionSpec("core"),) * n_outs
        self.sharded = jax.jit(
            shard_map(_body, mesh=self.mesh, in_specs=in_specs,
                      out_specs=out_specs, check_rep=False),
            donate_argnums=donate, keep_unused=True,
        )
        self.dev_in = None

    def put_inputs(self, in_maps):
        self.dev_in = [
            self.jax.device_put(
                np.concatenate([np.asarray(m[n]) for m in in_maps], axis=0), self.spec
            )
            for n in self.in_names
        ]

    def execute(self):
        zeros = [
            self.jax.device_put(
                np.zeros((CORES * z.shape[0], *z.shape[1:]), z.dtype), self.spec
            )
            for z in self.zero_outs
        ]
        for z in zeros:
            z.block_until_ready()
        t0 = time.monotonic_ns()
        outs = self.sharded(*self.dev_in, *zeros)
        for o in outs:
            o.block_until_ready()
        dt = time.monotonic_ns() - t0
        res = [
            {
                name: np.asarray(outs[i]).reshape(CORES, *self.zero_outs[i].shape)[c]
                for i, name in enumerate(self.out_names)
            }
            for c in range(CORES)
        ]
        return res, dt


def run_on_device(in_maps, K):
    if K not in _cache:
        _cache[K] = _Runner(_build(K))
    runner = _cache[K]
    runner.put_inputs(in_maps)
    res, dt = runner.execute()
    global LAST_EXEC_NS
    LAST_EXEC_NS = dt
    return res


def kernel(x, edge_index, W1, a_src1, a_dst1, b1, W2, a_src2, a_dst2, b2):
    global LAST_EXEC_NS
    K, in_maps = _preprocess(
        x, edge_index, W1, a_src1, a_dst1, b1, W2, a_src2, a_dst2, b2
    )
    res = run_on_device(in_maps, K)
    out = np.concatenate([res[c]["out"][:NPC] for c in range(CORES)], axis=0)
    return out.astype(np.float32)


# revision 5
# speedup vs baseline: 1.1566x; 1.1188x over previous
"""Two-layer GAT (GATConv 128->64x4 concat, relu, GATConv 256->2) on 8 TRN2
NeuronCores, self-contained.

v2: minimizes per-execute host->device upload (the dominant cost through the
axon tunnel) and device gather traffic.

Sharding: nodes are split 6250/core (padded to 6272 = 49 windows of 128).
Edges are partitioned by destination node; each core owns the edges whose dst
falls in its node range. Per core upload: xT shard fp16 [128, 6272] (1.6MB),
one int16 src-index stream shared by both layers (0.24MB, replicated to the 8
Q7 groups on device), uint8 slot stream (0.12MB), small weights.

Device pipeline per core:
  Phase A: h-slice[own 6272 nodes] = [x@W1 (256) | al_src (4) | al_dst (4) |
           pad] fp16, written to l1own; al_dst kept in SBUF per window.
  AllGather l1own -> l1full [50176, 384] fp16 (768B rows).
  Phase B: per window: dma_gather l1full rows by edge src; e = al_s[src] +
           al_d[dst] (dst side via PE-transposed one-hot matmul against the
           window's own al_d column); p = exp(leaky_relu(e));
           agg = onehot^T @ [h[src]*p | p] in PSUM; out1 = num/den + b1;
           relu; h2lite = relu1 @ [W2|W2 a_s2|W2 a_d2] -> l2own fp16.
  AllGather l2own -> l2full [50176, 128] fp16 (256B rows).
  Phase C: same window structure for layer 2; output [6272, 2] f32 rows.
"""

import os
import sys
import time

sys.path.insert(0, "/opt/trn_rl_repo")

import numpy as np

import concourse.bacc as bacc
import concourse.mybir as mybir
import concourse.tile as tile
from concourse.library_config import mlp
from concourse.masks import make_identity

# problem constants (hardcoded per harness contract)
N = 50000
INCH = 128
HID = 64
HEADS = 4
OUT = 2
NEG = 0.2
CORES = 8
NPC = N // CORES          # 6250 dst nodes per core
P = 128
W = 49                    # windows of 128 dst nodes per core (49*128 = 6272)
NPCP = W * P              # padded nodes per core (6272)
NROWS = CORES * NPCP      # table rows (50176)
T1C = 384                 # l1 table cols fp16 (768B rows): 0:256 h, 256:260
                          # al_src, 260:264 al_dst, rest zero
T2C = 128                 # l2 table cols fp16 (256B rows): 0:2 out, 2 al_s2,
                          # 3 al_d2, rest zero
BIAS = 32768              # int16 gather index bias
EPS = 1e-16
NQ = 4                    # SWDGE queues

f32 = mybir.dt.float32
f16 = mybir.dt.float16
i16 = mybir.dt.int16
i32 = mybir.dt.int32
u8 = mybir.dt.uint8

LAST_EXEC_NS = None
_cache = {}


def _wrap_idx_stream(arr):
    """arr [W, C] int16 -> [16, W*C//16] per-window 16-partition wrap."""
    Wn, C = arr.shape
    return arr.reshape(Wn, C // 16, 16).transpose(2, 0, 1).reshape(16, Wn * (C // 16)).copy()


def _chunks(K):
    """[(tile_off, ntiles)] with ntiles <= 8 (1024-idx dma_gather limit)."""
    out = []
    off = 0
    while off < K:
        n = min(8, K - off)
        out.append((off, n))
        off += n
    return out


def _build(K):
    C = K * P
    phases = os.environ.get("KPHASES", "ABGC")
    reps = int(os.environ.get("KREPS", "1"))
    nc = bacc.Bacc("TRN2", target_bir_lowering=False, debug=False,
                   num_devices=CORES, num_swdge_queues=NQ)

    # inputs
    xT_d = nc.dram_tensor("xT", [INCH, NPCP], f16, kind="ExternalInput")
    wcat_d = nc.dram_tensor("wcat", [INCH, T1C], f16, kind="ExternalInput")
    w2cat_d = nc.dram_tensor("w2cat", [P, 8], f16, kind="ExternalInput")
    b1_d = nc.dram_tensor("b1", [1, 256], f32, kind="ExternalInput")
    b2_d = nc.dram_tensor("b2", [1, 2], f32, kind="ExternalInput")
    idx16_d = nc.dram_tensor("idx16", [16, W * C // 16], i16, kind="ExternalInput")
    slots8_d = nc.dram_tensor("slots8", [P, W * K], u8, kind="ExternalInput")

    out_d = nc.dram_tensor("out", [NPCP, OUT], f32, kind="ExternalOutput")

    # scratch
    l1own = nc.dram_tensor("l1own", [NPCP, T1C], f16)
    l1full = nc.dram_tensor("l1full", [NROWS, T1C], f16, addr_space="Shared")
    l2own = nc.dram_tensor("l2own", [NPCP, T2C], f16)
    l2full = nc.dram_tensor("l2full", [NROWS, T2C], f16, addr_space="Shared")

    LR = mybir.AluOpType
    AF = mybir.ActivationFunctionType

    qctr = [0]

    def next_q():
        q = qctr[0] % NQ
        qctr[0] += 1
        return q

    with tile.TileContext(nc) as tc:
        with tc.tile_pool(name="const", bufs=1) as cpool:
            nc.gpsimd.load_library(mlp)

            ident_h = cpool.tile([P, P], f16)
            make_identity(nc, ident_h[:])
            ident_f = cpool.tile([P, P], f32)
            make_identity(nc, ident_f[:])
            iota_i = cpool.tile([P, P], i32)
            nc.gpsimd.iota(iota_i[:], pattern=[[1, P]], base=0, channel_multiplier=0)
            iota_h = cpool.tile([P, P], f16)
            nc.vector.tensor_copy(iota_h[:], iota_i[:])
            ones = cpool.tile([1, P], f32)
            nc.vector.memset(ones[:], 1.0)

            xT_sb = cpool.tile([INCH, NPCP], f16)
            nc.sync.dma_start(out=xT_sb[:], in_=xT_d[:, :])
            wcat_sb = cpool.tile([INCH, T1C], f16)
            nc.sync.dma_start(out=wcat_sb[:], in_=wcat_d[:, :])
            w2cat_sb = cpool.tile([P, 8], f16)
            nc.sync.dma_start(out=w2cat_sb[:], in_=w2cat_d[:, :])
            b1row = cpool.tile([1, 256], f32)
            nc.sync.dma_start(out=b1row[:], in_=b1_d[:, :])
            b2row = cpool.tile([1, 2], f32)
            nc.sync.dma_start(out=b2row[:], in_=b2_d[:, :])

            idx_sb = cpool.tile([P, W * C // 16], i16)
            for g in range(8):
                nc.sync.dma_start(
                    out=idx_sb[g * 16 : (g + 1) * 16, :], in_=idx16_d[:, :]
                )
            slots8_sb = cpool.tile([P, W * K], u8)
            nc.sync.dma_start(out=slots8_sb[:], in_=slots8_d[:, :])
            slots_f = cpool.tile([P, W * K], f32)
            nc.vector.tensor_copy(slots_f[:], slots8_sb[:])

            ald_all = cpool.tile([P, 4 * W], f16)
            ald2_all = cpool.tile([P, W], f16)
            ed2_all = cpool.tile([P, W * K], f16)
            l2stage = cpool.tile([P, T2C], f16)
            nc.vector.memset(l2stage[:], 0.0)

            # replicated biases
            with tc.tile_pool(name="psum_b", bufs=1, space="PSUM") as psb:
                b1_ps = psb.tile([P, 256], f32, space="PSUM")
                nc.tensor.matmul(out=b1_ps[:], lhsT=ones[:], rhs=b1row[:], start=True, stop=True)
                b1_rep = cpool.tile([P, 256], f32)
                nc.scalar.copy(b1_rep[:], b1_ps[:])
                b2_ps = psb.tile([P, 2], f32, space="PSUM")
                nc.tensor.matmul(out=b2_ps[:], lhsT=ones[:], rhs=b2row[:], start=True, stop=True)
                b2_rep = cpool.tile([P, 2], f32)
                nc.scalar.copy(b2_rep[:], b2_ps[:])

            for _rep in range(reps):
              # ---------------- Phase A: own-node features ----------------
              if "A" in phases:
                  with (
                      tc.tile_pool(name="sbufA", bufs=3) as pa,
                      tc.tile_pool(name="psumA", bufs=3, space="PSUM") as ppa,
                  ):
                      for w in range(W):
                          h_ps = ppa.tile([P, T1C], f32, space="PSUM", tag="h")
                          nc.tensor.matmul(
                              out=h_ps[:],
                              lhsT=xT_sb[:, w * P : (w + 1) * P],
                              rhs=wcat_sb[:],
                              start=True, stop=True,
                          )
                          stg = pa.tile([P, T1C], f16, tag="stg")
                          nc.scalar.copy(stg[:], h_ps[:])
                          nc.vector.tensor_copy(
                              ald_all[:, 4 * w : 4 * w + 4], h_ps[:, 260:264]
                          )
                          nc.sync.dma_start(
                              out=l1own[w * P : (w + 1) * P, :], in_=stg[:]
                          )

              # ---------------- AllGather layer-1 table ----------------
              if "G" in phases:
                  nc.gpsimd.collective_compute(
                      "AllGather",
                      mybir.AluOpType.bypass,
                      replica_groups=[list(range(CORES))],
                      ins=[l1own.ap().opt()],
                      outs=[l1full.ap().opt()],
                  )

              # ---------------- Phase B: layer-1 edge aggregation ----------------
              if "B" in phases:
                  with (
                      tc.tile_pool(name="sbufB", bufs=3) as pb,
                      tc.tile_pool(name="sbufBs", bufs=6) as pbs,
                      tc.tile_pool(name="psumAgg", bufs=2, space="PSUM") as pagg,
                      tc.tile_pool(name="psumT", bufs=2, space="PSUM") as pt,
                      tc.tile_pool(name="psumE", bufs=2, space="PSUM") as pe,
                      tc.tile_pool(name="psumH", bufs=1, space="PSUM") as ph,
                  ):
                      for w in range(W):
                          gbuf = pb.tile([P, K, T1C], f16, tag="gbuf")
                          ohTall = pb.tile([P, K, P], f16, tag="ohTall")
                          for (toff, ntl) in _chunks(K):
                              nc.gpsimd.dma_gather(
                                  gbuf[:, toff : toff + ntl, :],
                                  l1full[BIAS:, :],
                                  idx_sb[:, w * (C // 16) + toff * 8 : w * (C // 16) + (toff + ntl) * 8],
                                  ntl * P,
                                  ntl * P,
                                  T1C,
                                  queue_num=next_q(),
                              )
                          agg_ps = pagg.tile([P, 260], f32, space="PSUM", tag="agg")
                          for k in range(K):
                              onehot = pbs.tile([P, P], f16, tag="onehot")
                              nc.vector.tensor_scalar(
                                  out=onehot[:],
                                  in0=iota_h[:],
                                  scalar1=slots_f[:, w * K + k : w * K + k + 1],
                                  scalar2=None,
                                  op0=LR.is_equal,
                              )
                              ohT_ps = pt.tile([P, P], f16, space="PSUM", tag="trans")
                              nc.tensor.transpose(
                                  out=ohT_ps[:], in_=onehot[:], identity=ident_h[:]
                              )
                              nc.scalar.copy(ohTall[:, k, :], ohT_ps[:])
                              ed_ps = pe.tile([P, 4], f32, space="PSUM", tag="ed")
                              nc.tensor.matmul(
                                  out=ed_ps[:], lhsT=ohTall[:, k, :],
                                  rhs=ald_all[:, 4 * w : 4 * w + 4],
                                  start=True, stop=True,
                              )
                              e_sb = pbs.tile([P, 4], f32, tag="e")
                              nc.vector.tensor_copy(e_sb[:], gbuf[:, k, 256:260])
                              nc.vector.tensor_tensor(
                                  out=e_sb[:], in0=e_sb[:], in1=ed_ps[:], op=LR.add
                              )
                              lr_sb = pbs.tile([P, 4], f32, tag="lr")
                              nc.vector.scalar_tensor_tensor(
                                  out=lr_sb[:], in0=e_sb[:], scalar=NEG, in1=e_sb[:],
                                  op0=LR.mult, op1=LR.max,
                              )
                              p_sb = pbs.tile([P, 4], f32, tag="p")
                              nc.scalar.activation(p_sb[:], lr_sb[:], AF.Exp)
                              msg = pbs.tile([P, 260], f16, tag="msg")
                              for h in range(HEADS):
                                  nc.scalar.mul(
                                      msg[:, h * HID : (h + 1) * HID],
                                      gbuf[:, k, h * HID : (h + 1) * HID],
                                      p_sb[:, h : h + 1],
                                  )
                              nc.vector.tensor_copy(msg[:, 256:260], p_sb[:])
                              nc.tensor.matmul(
                                  out=agg_ps[:], lhsT=onehot[:], rhs=msg[:],
                                  start=(k == 0), stop=(k == K - 1),
                              )
                          # window readout
                          den = pbs.tile([P, 4], f32, tag="den")
                          nc.vector.tensor_scalar(
                              out=den[:], in0=agg_ps[:, 256:260], scalar1=EPS,
                              scalar2=None, op0=LR.add,
                          )
                          rec = pbs.tile([P, 4], f32, tag="rec")
                          nc.vector.reciprocal(rec[:], den[:])
                          relu1f = pbs.tile([P, 256], f32, tag="relu1f")
                          for h in range(HEADS):
                              nc.scalar.mul(
                                  relu1f[:, h * HID : (h + 1) * HID],
                                  agg_ps[:, h * HID : (h + 1) * HID],
                                  rec[:, h : h + 1],
                              )
                          nc.vector.tensor_tensor(
                              out=relu1f[:], in0=relu1f[:], in1=b1_rep[:], op=LR.add
                          )
                          relu1 = pbs.tile([P, 256], f16, tag="relu1")
                          nc.scalar.activation(relu1[:], relu1f[:], AF.Relu)
                          h2_ps = ph.tile([P, 4], f32, space="PSUM", tag="h2")
                          for half in range(2):
                              rT_ps = pt.tile([P, P], f16, space="PSUM", tag="trans")
                              nc.tensor.transpose(
                                  out=rT_ps[:], in_=relu1[:, half * P : (half + 1) * P],
                                  identity=ident_h[:],
                              )
                              rT = pbs.tile([P, P], f16, tag="rT")
                              nc.scalar.copy(rT[:], rT_ps[:])
                              nc.tensor.matmul(
                                  out=h2_ps[:], lhsT=rT[:],
                                  rhs=w2cat_sb[:, half * 4 : (half + 1) * 4],
                                  start=(half == 0), stop=(half == 1),
                              )
                          nc.vector.tensor_copy(l2stage[:, 0:4], h2_ps[:])
                          ald2_w = pbs.tile([P, 1], f16, tag="ald2w")
                          nc.vector.tensor_copy(ald2_w[:], h2_ps[:, 3:4])
                          nc.vector.tensor_copy(ald2_all[:, w : w + 1], ald2_w[:])
                          for k in range(K):
                              ed2_ps = pe.tile([P, 4], f32, space="PSUM", tag="ed")
                              nc.tensor.matmul(
                                  out=ed2_ps[:, 0:1], lhsT=ohTall[:, k, :],
                                  rhs=ald2_w[:],
                                  start=True, stop=True,
                              )
                              nc.vector.tensor_copy(
                                  ed2_all[:, w * K + k : w * K + k + 1], ed2_ps[:, 0:1]
                              )
                          nc.sync.dma_start(
                              out=l2own[w * P : (w + 1) * P, :], in_=l2stage[:]
                          )

              # ---------------- AllGather layer-2 table ----------------
              if "G" in phases:
                  nc.gpsimd.collective_compute(
                      "AllGather",
                      mybir.AluOpType.bypass,
                      replica_groups=[list(range(CORES))],
                      ins=[l2own.ap().opt()],
                      outs=[l2full.ap().opt()],
                  )

              # ---------------- Phase C: layer-2 edge aggregation ----------------
              if "C" in phases:
                  with (
                      tc.tile_pool(name="sbufC", bufs=3) as pc,
                      tc.tile_pool(name="sbufCs", bufs=6) as pcs,
                      tc.tile_pool(name="psumAgg2", bufs=2, space="PSUM") as pagg2,
                  ):
                      for w in range(W):
                          g2 = pc.tile([P, K, T2C], f16, tag="g2")
                          for (toff, ntl) in _chunks(K):
                              nc.gpsimd.dma_gather(
                                  g2[:, toff : toff + ntl, :],
                                  l2full[BIAS:, :],
                                  idx_sb[:, w * (C // 16) + toff * 8 : w * (C // 16) + (toff + ntl) * 8],
                                  ntl * P,
                                  ntl * P,
                                  T2C,
                                  queue_num=next_q(),
                              )
                          agg2_ps = pagg2.tile([P, 3], f32, space="PSUM", tag="agg2")
                          for k in range(K):
                              onehot = pcs.tile([P, P], f16, tag="onehot2")
                              nc.vector.tensor_scalar(
                                  out=onehot[:],
                                  in0=iota_h[:],
                                  scalar1=slots_f[:, w * K + k : w * K + k + 1],
                                  scalar2=None,
                                  op0=LR.is_equal,
                              )
                              e_sb = pcs.tile([P, 1], f32, tag="e2")
                              nc.vector.tensor_tensor(
                                  out=e_sb[:], in0=g2[:, k, 2:3],
                                  in1=ed2_all[:, w * K + k : w * K + k + 1], op=LR.add
                              )
                              lr_sb = pcs.tile([P, 1], f32, tag="lr2")
                              nc.vector.scalar_tensor_tensor(
                                  out=lr_sb[:], in0=e_sb[:], scalar=NEG, in1=e_sb[:],
                                  op0=LR.mult, op1=LR.max,
                              )
                              p_sb = pcs.tile([P, 1], f32, tag="p2")
                              nc.scalar.activation(p_sb[:], lr_sb[:], AF.Exp)
                              msg = pcs.tile([P, 3], f16, tag="msg2")
                              nc.scalar.mul(msg[:, 0:2], g2[:, k, 0:2], p_sb[:, 0:1])
                              nc.vector.tensor_copy(msg[:, 2:3], p_sb[:])
                              nc.tensor.matmul(
                                  out=agg2_ps[:], lhsT=onehot[:], rhs=msg[:],
                                  start=(k == 0), stop=(k == K - 1),
                              )
                          den = pcs.tile([P, 1], f32, tag="den2")
                          nc.vector.tensor_scalar(
                              out=den[:], in0=agg2_ps[:, 2:3], scalar1=EPS,
                              scalar2=None, op0=LR.add,
                          )
                          rec = pcs.tile([P, 1], f32, tag="rec2")
                          nc.vector.reciprocal(rec[:], den[:])
                          o2 = pcs.tile([P, OUT], f32, tag="o2")
                          nc.scalar.mul(o2[:], agg2_ps[:, 0:2], rec[:, 0:1])
                          nc.vector.tensor_tensor(out=o2[:], in0=o2[:], in1=b2_rep[:], op=LR.add)
                          nc.sync.dma_start(out=out_d[w * P : (w + 1) * P, :], in_=o2[:])

    nc.compile()
    return nc


def _preprocess(x, edge_index, W1, a_src1, a_dst1, b1, W2, a_src2, a_dst2, b2):
    src = np.concatenate([np.asarray(edge_index[0]), np.arange(N)]).astype(np.int64)
    dst = np.concatenate([np.asarray(edge_index[1]), np.arange(N)]).astype(np.int64)

    core = dst // NPC
    loc = dst - core * NPC
    win = loc >> 7
    slot = loc & 127
    gid = core * W + win
    order = np.argsort(gid, kind="stable")
    counts = np.bincount(gid, minlength=CORES * W)
    K = int(np.ceil((counts.max() + 1) / P))
    while counts.max() > K * P - len(_chunks(K)):
        K += 1
    C = K * P

    starts = np.zeros(CORES * W, np.int64)
    starts[1:] = np.cumsum(counts)[:-1]
    within = np.arange(len(order)) - starts[gid[order]]
    # map within-window rank -> position, skipping the reserved last slot of
    # each gather chunk (keeps every chunk's final index non-negative)
    usable = np.array([n * P - 1 for (_, n) in _chunks(K)], np.int64)
    cumu = np.cumsum(usable)
    ci = np.searchsorted(cumu, within, side="right")
    pos = gid[order] * C + within + ci

    s_sorted = src[order]
    perm = (s_sorted // NPC) * NPCP + (s_sorted % NPC)
    idx = np.full(CORES * W * C, BIAS, np.int64)
    idx[pos] = perm
    idx = (idx - BIAS).astype(np.int16).reshape(CORES, W, C)
    slotv = np.full(CORES * W * C, P, np.uint8)
    slotv[pos] = slot[order].astype(np.uint8)
    slotv = slotv.reshape(CORES, W, K, P)

    # weights
    W1 = np.asarray(W1, np.float32)
    W1r = W1.reshape(INCH, HEADS, HID)
    wa_s = np.einsum("ihc,hc->ih", W1r, np.asarray(a_src1, np.float32))
    wa_d = np.einsum("ihc,hc->ih", W1r, np.asarray(a_dst1, np.float32))
    wcat = np.zeros((INCH, T1C), np.float16)
    wcat[:, 0:256] = W1.astype(np.float16)
    wcat[:, 256:260] = wa_s.astype(np.float16)
    wcat[:, 260:264] = wa_d.astype(np.float16)

    W2 = np.asarray(W2, np.float32)
    w2s = W2 @ np.asarray(a_src2, np.float32)[0]
    w2d = W2 @ np.asarray(a_dst2, np.float32)[0]
    w2cat = np.concatenate([W2, w2s[:, None], w2d[:, None]], axis=1)
    w2cat = np.concatenate([w2cat[:P], w2cat[P:]], axis=1).astype(np.float16)  # [128, 8]

    x = np.asarray(x, np.float32)
    in_maps = []
    for c in range(CORES):
        xs = np.zeros((NPCP, INCH), np.float16)
        xs[:NPC] = x[c * NPC : (c + 1) * NPC].astype(np.float16)
        in_maps.append(
            {
                "xT": np.ascontiguousarray(xs.T),
                "wcat": wcat,
                "w2cat": w2cat,
                "b1": np.asarray(b1, np.float32).reshape(1, 256),
                "b2": np.asarray(b2, np.float32).reshape(1, 2),
                "idx16": _wrap_idx_stream(idx[c]),
                "slots8": slotv[c].transpose(2, 0, 1).reshape(P, W * K).copy(),
            }
        )
    return K, in_maps


class _Runner:
    """Persistent compiled runner: jit once, device-resident inputs, so
    repeated calls time only execution (+ dispatch)."""

    def __init__(self, nc):
        import jax
        from jax.sharding import Mesh, PartitionSpec, NamedSharding
        from jax.experimental.shard_map import shard_map
        from concourse import bass2jax
        import concourse.mybir as mb

        bass2jax.install_neuronx_cc_hook()
        self.jax = jax
        self.nc = nc
        part_name = nc.partition_id_tensor.name if nc.partition_id_tensor else None
        in_names, out_names, out_avals, zero_outs = [], [], [], []
        for alloc in nc.m.functions[0].allocations:
            if not isinstance(alloc, mb.MemoryLocationSet):
                continue
            name = alloc.memorylocations[0].name
            if alloc.kind == "ExternalInput":
                if name != part_name:
                    in_names.append(name)
            elif alloc.kind == "ExternalOutput":
                out_names.append(name)
                shape = tuple(alloc.tensor_shape)
                dtype = mb.dt.np(alloc.dtype)
                out_avals.append(jax.core.ShapedArray(shape, dtype))
                zero_outs.append(np.zeros(shape, dtype))
        self.in_names, self.out_names = in_names, out_names
        self.zero_outs = zero_outs
        n_params, n_outs = len(in_names), len(out_names)
        donate = tuple(range(n_params, n_params + n_outs))

        all_in_names = in_names + out_names + ([part_name] if part_name else [])

        def _body(*args):
            operands = list(args)
            if part_name is not None:
                operands.append(bass2jax.partition_id_tensor())
            outs = bass2jax._bass_exec_p.bind(
                *operands,
                out_avals=tuple(out_avals),
                in_names=tuple(all_in_names),
                out_names=tuple(out_names),
                lowering_input_output_aliases=(),
                sim_require_finite=True,
                sim_require_nnan=True,
                nc=nc,
            )
            return tuple(outs)

        devices = jax.devices()[:CORES]
        self.mesh = Mesh(np.asarray(devices), ("core",))
        self.spec = NamedSharding(self.mesh, PartitionSpec("core"))
        in_specs = (PartitionSpec("core"),) * (n_params + n_outs)
        out_specs = (PartitionSpec("core"),) * n_outs
        self.sharded = jax.jit(
            shard_map(_body, mesh=self.mesh, in_specs=in_specs,
                      out_specs=out_specs, check_rep=False),
            donate_argnums=donate, keep_unused=True,
        )
        self.dev_in = None

    def put_inputs(self, in_maps):
        self.dev_in = [
            self.jax.device_put(
                np.concatenate([np.asarray(m[n]) for m in in_maps], axis=0), self.spec
            )
            for n in self.in_names
        ]

    def execute(self):
        zeros = [
            self.jax.device_put(
                np.zeros((CORES * z.shape[0], *z.shape[1:]), z.dtype), self.spec
            )
            for z in self.zero_outs
        ]
        for z in zeros:
            z.block_until_ready()
        t0 = time.monotonic_ns()
        outs = self.sharded(*self.dev_in, *zeros)
        for o in outs:
            o.block_until_ready()
        dt = time.monotonic_ns() - t0
        res = [
            {
                name: np.asarray(outs[i]).reshape(CORES, *self.zero_outs[i].shape)[c]
                for i, name in enumerate(self.out_names)
            }
            for c in range(CORES)
        ]
        return res, dt


def run_on_device(in_maps, K):
    if K not in _cache:
        _cache[K] = _Runner(_build(K))
    runner = _cache[K]
    runner.put_inputs(in_maps)
    res, dt = runner.execute()
    global LAST_EXEC_NS
    LAST_EXEC_NS = dt
    return res


def kernel(x, edge_index, W1, a_src1, a_dst1, b1, W2, a_src2, a_dst2, b2):
    global LAST_EXEC_NS
    K, in_maps = _preprocess(
        x, edge_index, W1, a_src1, a_dst1, b1, W2, a_src2, a_dst2, b2
    )
    res = run_on_device(in_maps, K)
    out = np.concatenate([res[c]["out"][:NPC] for c in range(CORES)], axis=0)
    return out.astype(np.float32)
